# revision 49
# baseline (speedup 1.0000x reference)
"""Recursive LSTM decoder (T=512, B=512, I=128, H=512) on 8 trn2 NeuronCores.

Strategy: data-parallel over batch (64 rows/core, weights replicated, no
collectives). All on-chip state is kept in transposed layout
[feature-on-partition, batch-on-free] so the serial recurrence needs no
transposes. Matmul inputs are fp16 (1 cycle/row on PE, 10 mantissa bits);
accumulation and elementwise math are fp32; the cell state c stays fp32.

Per step (per core), gate order f, i, g, o:
  psg[j] = bias (K=4 matmul vs a chunk indicator, start=True)
           + sum_k Wcat.T-chunk(k,m) @ catT-chunk(k)  (16 m x 5 k, N=64)
  tanh-only gates: sg(z)=(tanh(z/2)+1)/2 with state C=2c, H=2h (W_hh/fc_W
  host-halved, g-gate rows host-doubled), so every gate is tanh(0.5 psum)
  and the c/h chain is 3 DVE stt ops + 1 ACT tanh, half-split so the
  ACT->DVE chain pipelines and mostly hides under the o-gate matmuls.
  feedback: in' = tanh(0.5*(fcW.T @ hT) + fc_b/2) = next step's input AND
  the reference output of this step TRANSPOSED ([i, b]); it is DMA'd as-is
  to DRAM rows [(T-1-t)*I, (T-t)*I) of a [T*I, BS] fp16 tensor (reference
  stores outputs reversed) and the host untransposes + casts to fp32.

The loop is unrolled 8 steps per For_i iteration to amortize the
staggered_reset semaphore barrier; 4 rotating feedback buffers give each
output-store DMA 4 steps of slack. Host<->device traffic is minimized:
weights/biases upload once and broadcast device-side (replicated
PartitionSpec), per-core state is sharded, donated output buffers are
created on-device, and the output downloads as fp16.
"""

import numpy as np
import ml_dtypes

import concourse.bass as bass
import concourse.mybir as mybir
import concourse.tile as tile
from concourse import bacc
from concourse.bass import ds
from concourse.expressions import smax

T, B, I, H = 512, 512, 128, 512
NCORES = 8
BS = B // NCORES          # 64 batch rows per core
HC = H // 128             # 4 h chunks
NM = (4 * H) // 128       # 16 gate m-chunks
NK = (I + H) // 128       # 5 cat k-chunks (1 input + 4 hidden)

# bf16 WEIGHT bundle (identical on every core -> replicated upload)
OFF_WG = 0                       # [128, NM*NK*128] gate weight chunks
OFF_WFC = OFF_WG + NM * NK * 128  # [128, HC*128] fc weight chunks
OFF_FCBR = OFF_WFC + HC * 128    # [1, 128] fc bias row (row 0 only)
OFF_BMM = OFF_FCBR + 128         # [4, 4*128] gate bias lhsT (part=chunk c)
OFF_IND = OFF_BMM + 4 * 128      # [4, HC*BS] chunk indicator rhs
WT_COLS = OFF_IND + HC * BS
# bf16 per-core STATE bundle
OFF_XT = 0                       # [128, BS] x[T-1] transposed
OFF_H0 = OFF_XT + BS             # [128, HC*BS] h0 transposed
ST_COLS = OFF_H0 + HC * BS
# f32 BIAS bundle (identical on every core -> replicated upload)
OFF_FCBH = 0                     # [128, 1] fc_b / 2
BB_COLS = OFF_FCBH + 1
# f32 per-core c0 bundle: [128, HC*BS]
C0_COLS = HC * BS

BF16 = mybir.dt.bfloat16
F16 = mybir.dt.float16
F32 = mybir.dt.float32
AF = mybir.ActivationFunctionType


def build(nsteps: int, out_steps: int | None = None, repeat: int = 1):
    """repeat>1 is a timing mode: the loop runs nsteps*repeat steps; stores
    for t >= nsteps fall out of range and are skipped via bounds_check."""
    out_steps = out_steps or nsteps
    nc = bacc.Bacc()
    wts16 = nc.dram_tensor("wts16", [128, WT_COLS], F16, kind="ExternalInput")
    st16 = nc.dram_tensor("st16", [128, ST_COLS], F16, kind="ExternalInput")
    bb32 = nc.dram_tensor("bb32", [128, BB_COLS], F32, kind="ExternalInput")
    c032 = nc.dram_tensor("c032", [128, C0_COLS], F32, kind="ExternalInput")
    # output stored TRANSPOSED per step: step t -> rows [(T-1-t)*I, (T-t)*I)
    # of [steps*I, BS]; out[t] == in_{t+1}.T exactly (same tanh), so the
    # feedback tile doubles as the output and the host untransposes.
    out = nc.dram_tensor("out", [out_steps * I, BS], F16, kind="ExternalOutput")

    with tile.TileContext(nc) as tc:
        with (
            tc.tile_pool(name="consts", bufs=1) as consts,
            tc.tile_pool(name="state", bufs=1) as state,
            tc.tile_pool(name="gact", bufs=3) as gact,
            tc.tile_pool(name="psst", bufs=1, space="PSUM") as psst,
            tc.tile_pool(name="pf", bufs=2, space="PSUM") as pfp,
        ):
            CW = consts.tile([128, WT_COLS], F16)
            nc.sync.dma_start(out=CW, in_=wts16[:])
            CS = consts.tile([128, ST_COLS], F16)
            nc.sync.dma_start(out=CS, in_=st16[:])
            CB = consts.tile([128, BB_COLS], F32)
            nc.sync.dma_start(out=CB, in_=bb32[:])
            CC = consts.tile([128, C0_COLS], F32)
            nc.sync.dma_start(out=CC, in_=c032[:])

            def wg_chunk(m, k):
                o = OFF_WG + (m * NK + k) * 128
                return CW[:, o:o + 128]

            def wfc_chunk(k):
                o = OFF_WFC + k * 128
                return CW[:, o:o + 128]

            bias_mm = CW[0:4, OFF_BMM:OFF_BMM + 4 * 128]
            ind_mm = CW[0:4, OFF_IND:OFF_IND + HC * BS]
            fb_h = CB[:, OFF_FCBH:OFF_FCBH + 1]

            hA = state.tile([128, HC, BS], F16)
            nc.vector.tensor_copy(
                hA, CS[:, OFF_H0:OFF_H0 + HC * BS].rearrange(
                    "p (c b) -> p c b", c=HC))
            hB = state.tile([128, HC, BS], F16)
            cT = state.tile([128, HC, BS], F32)
            nc.vector.tensor_copy(
                cT, CC[:, 0:HC * BS].rearrange("p (c b) -> p c b", c=HC))
            # 4 rotating feedback/output buffers: step u reads ins[(u+3)%4],
            # writes ins[u%4]; the store DMA of a buffer gets 4 steps of
            # slack before the buffer is rewritten. x[T-1] seeds ins[3].
            ins = [state.tile([128, BS], F16, name=f"in{q}")
                   for q in range(4)]
            nc.vector.tensor_copy(ins[3], CS[:, OFF_XT:OFF_XT + BS])
            # prologue tanh so the ACT table set is loaded on every path into
            # the loop -- otherwise the table-load lands INSIDE the body
            warm = state.tile([128, 1], F32)
            nc.scalar.activation(warm, CB[:, OFF_FCBH:OFF_FCBH + 1], AF.Tanh)

            # persistent per-gate PSUM accumulators [p, h-chunk, b]; each
            # step's first write is a start=True K=4 bias matmul (chunk
            # indicator x per-chunk bias row), so no DVE seeding needed
            psg = [psst.tile([128, HC, BS], F32, name=f"psg{j}")
                   for j in range(4)]

            cTf = cT.rearrange("p c b -> p (c b)")
            psgf = [p.rearrange("p c b -> p (c b)") for p in psg]

            def step(t, h_in, h_out, in_in, in_out):
                # Per-gate PSUM: psg[j] holds gate j for all 4 H-chunks.
                # DVE pre-writes the bias into the bank; matmuls accumulate
                # on top (start=False, has_written set in prologue).
                # Gate order f, i, g, o: u_s=(th_f+1)*C can run right after
                # the first ACT, and the v_s/cTf/tanh_c chain hides under
                # the o-gate matmuls; only tanh_o -> h remains in the tail.
                # sigmoid-free: sg(z)=(tanh(z/2)+1)/2, state C=2c, H=2h
                # (W_hh, fc_W host-halved; g-gate weights/bias host-doubled
                # so every gate uses tanh(0.5*psum)).
                th = {}
                v_s = gact.tile([128, HC * BS], F32, tag="v_s")
                u_s = gact.tile([128, HC * BS], F32, tag="u_s")
                tc_s = gact.tile([128, HC * BS], F32, tag="tc_s")
                HB2 = HC * BS // 2  # half-split of the c/h chain ops

                def half(x, s):
                    return x[:, s * HB2:(s + 1) * HB2]

                for j in (1, 0, 2, 3):
                    nc.tensor.matmul(
                        psgf[j], lhsT=bias_mm[:, j * 128:(j + 1) * 128],
                        rhs=ind_mm, start=True, stop=False,
                        skip_group_check=True)
                    for c in range(HC):
                        m = j * 4 + c
                        for k in (1, 2, 3, 4, 0):
                            mv = in_in if k == 0 else h_in[:, k - 1, :]
                            nc.tensor.matmul(
                                psg[j][:, c, :], lhsT=wg_chunk(m, k), rhs=mv,
                                start=False, stop=(k == 0),
                                skip_group_check=True)
                    th_j = gact.tile([128, HC * BS], F32, tag=f"th{j}")
                    th[j] = th_j
                    if j in (2, 3):
                        # on the critical chain: half-split ACT so the DVE
                        # consumers pipeline behind the first half
                        for s in (0, 1):
                            nc.scalar.activation(half(th_j, s),
                                                 half(psgf[j], s),
                                                 AF.Tanh, scale=0.5)
                    else:
                        nc.scalar.activation(th_j, psgf[j], AF.Tanh,
                                             scale=0.5)
                    if j == 1:
                        # A=(th_f+1)*C=4fc
                        nc.vector.scalar_tensor_tensor(
                            u_s, th[1], 1.0, cTf,
                            op0=mybir.AluOpType.add, op1=mybir.AluOpType.mult)
                    elif j == 2:
                        # B=(th_i+1)*g=2ig, C_new=A/2+B=2c_new
                        for s in (0, 1):
                            nc.vector.scalar_tensor_tensor(
                                half(v_s, s), half(th[0], s), 1.0,
                                half(th[2], s),
                                op0=mybir.AluOpType.add,
                                op1=mybir.AluOpType.mult)
                            nc.vector.scalar_tensor_tensor(
                                half(cTf, s), half(u_s, s), 0.5,
                                half(v_s, s),
                                op0=mybir.AluOpType.mult,
                                op1=mybir.AluOpType.add)
                            nc.scalar.activation(half(tc_s, s),
                                                 half(cTf, s),
                                                 AF.Tanh, scale=0.5)
                # H = (th_o+1)*tanh(c) = 2h
                hof = h_out.rearrange("p c b -> p (c b)")
                for s in (0, 1):
                    nc.vector.scalar_tensor_tensor(
                        half(hof, s), half(th[3], s), 1.0, half(tc_s, s),
                        op0=mybir.AluOpType.add, op1=mybir.AluOpType.mult)

                # feedback fc: in_out = tanh(0.5*fc(h) + fc_b/2) [128 i, BS b]
                # in_out IS the reference output of step t transposed, so it
                # is DMA'd directly; the host untransposes.
                pf = pfp.tile([128, BS], F32, tag="pf")
                for k in range(HC):
                    nc.tensor.matmul(pf, lhsT=wfc_chunk(k), rhs=h_out[:, k, :],
                                     start=(k == 0), stop=(k == HC - 1))
                nc.scalar.activation(in_out, pf, AF.Tanh, bias=fb_h, scale=0.5)
                # repeat>1 (timing mode): extra steps clamp to row 0 (junk)
                row = (nsteps - 1 - t) * I
                if repeat > 1:
                    row = smax(0, row)
                nc.sync.dma_start(out=out[ds(row, I), :], in_=in_out)

            unroll = 8 if (nsteps * repeat) % 8 == 0 else 2
            with tc.For_i(0, nsteps * repeat, unroll,
                          staggered_reset=True) as t:
                if unroll == 2:
                    step(t, hA, hB, ins[3], ins[0])
                    step(t + 1, hB, hA, ins[0], ins[3])
                else:
                    for u in range(0, unroll, 2):
                        step(t + u, hA, hB, ins[(u + 3) % 4], ins[u % 4])
                        step(t + u + 1, hB, hA, ins[u % 4],
                             ins[(u + 1) % 4])

    nc.finalize()
    return nc


_cache = {}


def _get_nc(nsteps, out_steps=None, repeat=1):
    key = (nsteps, out_steps, repeat)
    if key not in _cache:
        _cache[key] = build(nsteps, out_steps, repeat)
    return _cache[key]


def _prep_inputs(x, h0, c0, W_ih, W_hh, b_ih, b_hh, fc_W, fc_b, nsteps):
    """-> (shared_map {name: array}, percore_map {name: (8*128, cols) array})."""
    f32 = np.float32
    bf16 = ml_dtypes.bfloat16
    x = np.asarray(x, f32)
    h0 = np.asarray(h0, f32)
    c0 = np.asarray(c0, f32)
    # state is H=2h, C=2c with W_hh/fc_W halved to compensate; g-gate rows
    # doubled so all gates share tanh(0.5*(psum)) with psum pre-biased
    W_cat = np.concatenate(
        [np.asarray(W_ih, f32), 0.5 * np.asarray(W_hh, f32)], axis=1)
    W_cat[1024:1536, :] *= 2.0
    wg_np = W_cat.reshape(NM, 128, NK, 128).transpose(3, 0, 2, 1).reshape(
        128, NM * NK * 128)
    fc_W = np.asarray(fc_W, f32)
    wfc_np = (0.5 * fc_W).reshape(I, HC, 128).transpose(2, 1, 0).reshape(
        128, HC * 128)
    b = np.asarray(b_ih, f32) + np.asarray(b_hh, f32)
    badj = b.copy()
    badj[1024:1536] *= 2.0
    fc_b = np.asarray(fc_b, f32)

    wts = np.zeros((128, WT_COLS), f32)
    wts[:, OFF_WG:OFF_WG + NM * NK * 128] = wg_np
    wts[:, OFF_WFC:OFF_WFC + HC * 128] = wfc_np
    wts[0, OFF_FCBR:OFF_FCBR + 128] = fc_b
    # gate-bias lhsT [part=chunk c, col=j*128+p] = badj[j, c, p]
    wts[0:4, OFF_BMM:OFF_BMM + 4 * 128] = badj.reshape(
        4, HC, 128).transpose(1, 0, 2).reshape(HC, 4 * 128)
    wts[0:4, OFF_IND:OFF_IND + HC * BS] = np.repeat(np.eye(HC, dtype=f32),
                                                    BS, axis=1)

    bb = np.zeros((128, BB_COLS), f32)
    bb[:, OFF_FCBH] = 0.5 * fc_b

    # per-core bundles, stacked core-major for PartitionSpec("core")
    st = np.zeros((NCORES, 128, ST_COLS), f32)
    cc = np.zeros((NCORES, 128, C0_COLS), f32)
    for core in range(NCORES):
        sl = slice(core * BS, (core + 1) * BS)
        st[core, :, OFF_XT:OFF_XT + BS] = x[nsteps - 1, sl, :].T
        st[core, :, OFF_H0:OFF_H0 + HC * BS] = 2.0 * h0[0, sl, :].reshape(
            BS, HC, 128).transpose(2, 1, 0).reshape(128, -1)
        cc[core] = 2.0 * c0[0, sl, :].reshape(
            BS, HC, 128).transpose(2, 1, 0).reshape(128, -1)
    shared = {
        "wts16": np.ascontiguousarray(wts).astype(np.float16),
        "bb32": np.ascontiguousarray(bb),
    }
    percore = {
        "st16": np.ascontiguousarray(st.astype(np.float16)).reshape(
            NCORES * 128, ST_COLS),
        "c032": np.ascontiguousarray(cc).reshape(NCORES * 128, C0_COLS),
    }
    return shared, percore


class _Runner:
    """Cached jitted 8-core executor for one build() configuration.

    Bypasses run_bass_kernel_spmd so repeated calls reuse the jitted
    callable (no re-trace / NEFF reload) and so the donated output
    buffers are created ON DEVICE (jnp.zeros jit) instead of being
    shipped over the axon tunnel (134 MB of zeros at fp32).
    """

    def __init__(self, nsteps, out_steps=None, repeat=1):
        import jax
        import jax.numpy as jnp
        from jax.sharding import Mesh, PartitionSpec, NamedSharding
        from concourse.bass2jax import (
            _bass_exec_p, install_neuronx_cc_hook, partition_id_tensor,
            shard_map)

        self.jax = jax
        self.nsteps = nsteps
        self.out_steps = out_steps or nsteps
        nc = _get_nc(nsteps, out_steps, repeat)
        self.nc = nc
        install_neuronx_cc_hook()

        part_name = (nc.partition_id_tensor.name
                     if nc.partition_id_tensor else None)
        in_names, out_names, out_avals = [], [], []
        for alloc in nc.m.functions[0].allocations:
            if not isinstance(alloc, mybir.MemoryLocationSet):
                continue
            name = alloc.memorylocations[0].name
            if alloc.kind == "ExternalInput":
                if name != part_name:
                    in_names.append(name)
            elif alloc.kind == "ExternalOutput":
                out_names.append(name)
                out_avals.append(jax.core.ShapedArray(
                    tuple(alloc.tensor_shape), mybir.dt.np(alloc.dtype)))
        self.in_names = in_names
        self.out_names = out_names
        self.out_avals = out_avals
        self.shared_names = ("wts16", "bb32")
        n_params = len(in_names)
        n_outs = len(out_avals)
        all_names = tuple(in_names) + tuple(out_names)
        if part_name is not None:
            all_names = all_names + (part_name,)

        def _body(*args):
            operands = list(args)
            if part_name is not None:
                operands.append(partition_id_tensor())
            outs = _bass_exec_p.bind(
                *operands, out_avals=tuple(out_avals),
                in_names=all_names, out_names=tuple(out_names),
                lowering_input_output_aliases=(),
                sim_require_finite=True, sim_require_nnan=True, nc=nc)
            return tuple(outs)

        devices = jax.devices()[:NCORES]
        self.dev0 = devices[0]
        mesh = Mesh(np.asarray(devices), ("core",))
        self.sharding = NamedSharding(mesh, PartitionSpec("core"))
        self.rep_sharding = NamedSharding(mesh, PartitionSpec())
        in_specs = tuple(
            PartitionSpec() if n in self.shared_names else
            PartitionSpec("core") for n in in_names
        ) + (PartitionSpec("core"),) * n_outs
        out_specs = (PartitionSpec("core"),) * n_outs
        donate = tuple(range(n_params, n_params + n_outs))
        self.sharded = jax.jit(
            shard_map(_body, mesh=mesh, in_specs=in_specs,
                      out_specs=out_specs, check_rep=False),
            donate_argnums=donate, keep_unused=True)
        zero_shapes = [(NCORES * a.shape[0], *a.shape[1:]) for a in out_avals]
        zero_dtypes = [a.dtype for a in out_avals]
        self.mkzeros = jax.jit(
            lambda: tuple(jnp.zeros(s, d)
                          for s, d in zip(zero_shapes, zero_dtypes)),
            out_shardings=tuple(self.sharding for _ in out_avals))

    def device_put_inputs(self, shared, percore):
        """Upload: shared tensors once + device-side broadcast; per-core
        tensors sharded. Returns the operand list in in_names order."""
        jax = self.jax
        arrs = []
        for name in self.in_names:
            if name in self.shared_names:
                d0 = jax.device_put(shared[name],
                                    jax.sharding.SingleDeviceSharding(
                                        self.dev0))
                arrs.append(jax.device_put(d0, self.rep_sharding))
            else:
                arrs.append(jax.device_put(percore[name], self.sharding))
        jax.block_until_ready(arrs)
        return arrs

    def execute(self, operands):
        outs = self.sharded(*operands, *self.mkzeros())
        self.jax.block_until_ready(outs)
        return outs

    def fetch(self, outs):
        return {name: np.asarray(o) for name, o in zip(self.out_names, outs)}


def jax_block(x):
    import jax
    jax.block_until_ready(x)


_runners = {}


def _get_runner(nsteps, out_steps=None, repeat=1) -> _Runner:
    key = (nsteps, out_steps, repeat)
    if key not in _runners:
        _runners[key] = _Runner(nsteps, out_steps, repeat)
    return _runners[key]


def _assemble(out_global: np.ndarray, steps: int) -> np.ndarray:
    """(8*steps*I, BS) core-major transposed output -> (steps, B, I) fp32."""
    per_core = out_global.reshape(NCORES, steps, I, BS)
    return np.ascontiguousarray(
        per_core.transpose(1, 0, 3, 2)).reshape(steps, B, I).astype(
        np.float32)


def run(x, h0, c0, W_ih, W_hh, b_ih, b_hh, fc_W, fc_b, nsteps=T,
        out_steps=None, repeat=1):
    r = _get_runner(nsteps, out_steps, repeat)
    shared, percore = _prep_inputs(x, h0, c0, W_ih, W_hh, b_ih, b_hh,
                                   fc_W, fc_b, nsteps)
    outs = r.execute(r.device_put_inputs(shared, percore))
    host = r.fetch(outs)
    return _assemble(host["out"], out_steps or nsteps), None


def kernel(x, enc_hiddens, h0, c0, W_ih, W_hh, b_ih, b_hh, fc_W, fc_b):
    outs, _ = run(x, h0, c0, W_ih, W_hh, b_ih, b_hh, fc_W, fc_b, nsteps=T)
    return outs



# revision 56
# speedup vs baseline: 1.1354x; 1.1354x over previous
"""Recursive LSTM decoder (T=512, B=512, I=128, H=512) on 8 trn2 NeuronCores.

Strategy: data-parallel over batch (64 rows/core, weights replicated, no
collectives). All on-chip state is kept in transposed layout
[feature-on-partition, batch-on-free] so the serial recurrence needs no
transposes. Matmul inputs are fp16 (1 cycle/row on PE, 10 mantissa bits);
accumulation and elementwise math are fp32; the cell state c stays fp32.

Per step (per core), gate order f, i, g, o:
  psg[j] = bias (K=4 matmul vs a chunk indicator, start=True)
           + sum_k Wcat.T-chunk(k,m) @ catT-chunk(k)  (16 m x 5 k, N=64)
  tanh-only gates: sg(z)=(tanh(z/2)+1)/2 with state C=2c, H=2h (W_hh/fc_W
  host-halved, g-gate rows host-doubled), so every gate is tanh(0.5 psum)
  and the c/h chain is 3 DVE stt ops + 1 ACT tanh, half-split so the
  ACT->DVE chain pipelines and mostly hides under the o-gate matmuls.
  feedback: in' = tanh(0.5*(fcW.T @ hT) + fc_b/2)   [= 2*sigmoid(z)-1]
  output:   out[64,128] = tanh(0.5*(hT stationary @ fcW-moving + fc_b)),
  stored fp16 at DRAM row (T-1-t)*BS (reference stores outputs reversed).

The loop is unrolled 8 steps per For_i iteration to amortize the
staggered_reset semaphore barrier; 4 rotating feedback buffers give each
output-store DMA 4 steps of slack. Host<->device traffic is minimized:
weights/biases upload once and broadcast device-side (replicated
PartitionSpec), per-core state is sharded, donated output buffers are
created on-device, and the output downloads as fp16.
"""

import numpy as np
import ml_dtypes

import concourse.bass as bass
import concourse.mybir as mybir
import concourse.tile as tile
from concourse import bacc
from concourse.bass import ds
from concourse.expressions import smax

T, B, I, H = 512, 512, 128, 512
NCORES = 8
BS = B // NCORES          # 64 batch rows per core
HC = H // 128             # 4 h chunks
NM = (4 * H) // 128       # 16 gate m-chunks
NK = (I + H) // 128       # 5 cat k-chunks (1 input + 4 hidden)

# bf16 WEIGHT bundle (identical on every core -> replicated upload)
OFF_WG = 0                       # [128, NM*NK*128] gate weight chunks
OFF_WFC = OFF_WG + NM * NK * 128  # [128, HC*128] fc weight chunks
OFF_FCBR = OFF_WFC + HC * 128    # [1, 128] fc bias row (row 0 only)
OFF_BMM = OFF_FCBR + 128         # [4, 4*128] gate bias lhsT (part=chunk c)
OFF_IND = OFF_BMM + 4 * 128      # [4, HC*BS] chunk indicator rhs
WT_COLS = OFF_IND + HC * BS
# bf16 per-core STATE bundle
OFF_XT = 0                       # [128, BS] x[T-1] transposed
OFF_H0 = OFF_XT + BS             # [128, HC*BS] h0 transposed
ST_COLS = OFF_H0 + HC * BS
# f32 BIAS bundle (identical on every core -> replicated upload)
OFF_FCBH = 0                     # [128, 1] fc_b / 2
BB_COLS = OFF_FCBH + 1
# f32 per-core c0 bundle: [128, HC*BS]
C0_COLS = HC * BS

BF16 = mybir.dt.bfloat16
F16 = mybir.dt.float16
F32 = mybir.dt.float32
AF = mybir.ActivationFunctionType


def build(nsteps: int, out_steps: int | None = None, repeat: int = 1):
    """repeat>1 is a timing mode: the loop runs nsteps*repeat steps; stores
    for t >= nsteps fall out of range and are skipped via bounds_check."""
    out_steps = out_steps or nsteps
    nc = bacc.Bacc()
    wts16 = nc.dram_tensor("wts16", [128, WT_COLS], F16, kind="ExternalInput")
    st16 = nc.dram_tensor("st16", [128, ST_COLS], F16, kind="ExternalInput")
    bb32 = nc.dram_tensor("bb32", [128, BB_COLS], F32, kind="ExternalInput")
    c032 = nc.dram_tensor("c032", [128, C0_COLS], F32, kind="ExternalInput")
    out = nc.dram_tensor("out", [out_steps * BS, I], F16,
                         kind="ExternalOutput")

    with tile.TileContext(nc) as tc:
        with (
            tc.tile_pool(name="consts", bufs=1) as consts,
            tc.tile_pool(name="state", bufs=1) as state,
            tc.tile_pool(name="gact", bufs=3) as gact,
            tc.tile_pool(name="outp", bufs=3) as outp,
            tc.tile_pool(name="psst", bufs=1, space="PSUM") as psst,
            tc.tile_pool(name="pf", bufs=2, space="PSUM") as pfp,
            tc.tile_pool(name="po", bufs=2, space="PSUM") as pop,
        ):
            CW = consts.tile([128, WT_COLS], F16)
            nc.sync.dma_start(out=CW, in_=wts16[:])
            CS = consts.tile([128, ST_COLS], F16)
            nc.sync.dma_start(out=CS, in_=st16[:])
            CB = consts.tile([128, BB_COLS], F32)
            nc.sync.dma_start(out=CB, in_=bb32[:])
            CC = consts.tile([128, C0_COLS], F32)
            nc.sync.dma_start(out=CC, in_=c032[:])
            ones = consts.tile([1, HC * BS], F16)
            nc.vector.memset(ones, 1.0)

            def wg_chunk(m, k):
                o = OFF_WG + (m * NK + k) * 128
                return CW[:, o:o + 128]

            def wfc_chunk(k):
                o = OFF_WFC + k * 128
                return CW[:, o:o + 128]

            fb_r = CW[0:1, OFF_FCBR:OFF_FCBR + 128]
            bias_mm = CW[0:4, OFF_BMM:OFF_BMM + 4 * 128]
            ind_mm = CW[0:4, OFF_IND:OFF_IND + HC * BS]
            fb_h = CB[:, OFF_FCBH:OFF_FCBH + 1]

            hA = state.tile([128, HC, BS], F16)
            nc.vector.tensor_copy(
                hA, CS[:, OFF_H0:OFF_H0 + HC * BS].rearrange(
                    "p (c b) -> p c b", c=HC))
            hB = state.tile([128, HC, BS], F16)
            cT = state.tile([128, HC, BS], F32)
            nc.vector.tensor_copy(
                cT, CC[:, 0:HC * BS].rearrange("p (c b) -> p c b", c=HC))
            # 4 rotating feedback/output buffers: step u reads ins[(u+3)%4],
            # writes ins[u%4]; the store DMA of a buffer gets 4 steps of
            # slack before the buffer is rewritten. x[T-1] seeds ins[3].
            ins = [state.tile([128, BS], F16, name=f"in{q}")
                   for q in range(4)]
            nc.vector.tensor_copy(ins[3], CS[:, OFF_XT:OFF_XT + BS])
            # prologue tanh so the ACT table set is loaded on every path into
            # the loop -- otherwise the table-load lands INSIDE the body
            warm = state.tile([128, 1], F32)
            nc.scalar.activation(warm, CB[:, OFF_FCBH:OFF_FCBH + 1], AF.Tanh)

            # persistent per-gate PSUM accumulators [p, h-chunk, b]; each
            # step's first write is a start=True K=4 bias matmul (chunk
            # indicator x per-chunk bias row), so no DVE seeding needed
            psg = [psst.tile([128, HC, BS], F32, name=f"psg{j}")
                   for j in range(4)]

            cTf = cT.rearrange("p c b -> p (c b)")
            psgf = [p.rearrange("p c b -> p (c b)") for p in psg]

            def step(t, h_in, h_out, in_in, in_out):
                # Per-gate PSUM: psg[j] holds gate j for all 4 H-chunks.
                # DVE pre-writes the bias into the bank; matmuls accumulate
                # on top (start=False, has_written set in prologue).
                # Gate order f, i, g, o: u_s=(th_f+1)*C can run right after
                # the first ACT, and the v_s/cTf/tanh_c chain hides under
                # the o-gate matmuls; only tanh_o -> h remains in the tail.
                # sigmoid-free: sg(z)=(tanh(z/2)+1)/2, state C=2c, H=2h
                # (W_hh, fc_W host-halved; g-gate weights/bias host-doubled
                # so every gate uses tanh(0.5*psum)).
                th = {}
                v_s = gact.tile([128, HC * BS], F32, tag="v_s")
                u_s = gact.tile([128, HC * BS], F32, tag="u_s")
                tc_s = gact.tile([128, HC * BS], F32, tag="tc_s")
                HB2 = HC * BS // 2  # half-split of the c/h chain ops

                def half(x, s):
                    return x[:, s * HB2:(s + 1) * HB2]

                for j in (1, 0, 2, 3):
                    nc.tensor.matmul(
                        psgf[j], lhsT=bias_mm[:, j * 128:(j + 1) * 128],
                        rhs=ind_mm, start=True, stop=False,
                        skip_group_check=True)
                    for c in range(HC):
                        m = j * 4 + c
                        for k in (1, 2, 3, 4, 0):
                            mv = in_in if k == 0 else h_in[:, k - 1, :]
                            nc.tensor.matmul(
                                psg[j][:, c, :], lhsT=wg_chunk(m, k), rhs=mv,
                                start=False, stop=(k == 0),
                                skip_group_check=True)
                    th_j = gact.tile([128, HC * BS], F32, tag=f"th{j}")
                    th[j] = th_j
                    if j in (2, 3):
                        # on the critical chain: half-split ACT so the DVE
                        # consumers pipeline behind the first half
                        for s in (0, 1):
                            nc.scalar.activation(half(th_j, s),
                                                 half(psgf[j], s),
                                                 AF.Tanh, scale=0.5)
                    else:
                        nc.scalar.activation(th_j, psgf[j], AF.Tanh,
                                             scale=0.5)
                    if j == 1:
                        # A=(th_f+1)*C=4fc
                        nc.vector.scalar_tensor_tensor(
                            u_s, th[1], 1.0, cTf,
                            op0=mybir.AluOpType.add, op1=mybir.AluOpType.mult)
                    elif j == 2:
                        # B=(th_i+1)*g=2ig, C_new=A/2+B=2c_new
                        for s in (0, 1):
                            nc.vector.scalar_tensor_tensor(
                                half(v_s, s), half(th[0], s), 1.0,
                                half(th[2], s),
                                op0=mybir.AluOpType.add,
                                op1=mybir.AluOpType.mult)
                            nc.vector.scalar_tensor_tensor(
                                half(cTf, s), half(u_s, s), 0.5,
                                half(v_s, s),
                                op0=mybir.AluOpType.mult,
                                op1=mybir.AluOpType.add)
                            nc.scalar.activation(half(tc_s, s),
                                                 half(cTf, s),
                                                 AF.Tanh, scale=0.5)
                # H = (th_o+1)*tanh(c) = 2h
                hof = h_out.rearrange("p c b -> p (c b)")
                for s in (0, 1):
                    nc.vector.scalar_tensor_tensor(
                        half(hof, s), half(th[3], s), 1.0, half(tc_s, s),
                        op0=mybir.AluOpType.add, op1=mybir.AluOpType.mult)

                # feedback fc: in_out = tanh(0.5*fc(h) + fc_b/2) [128 i, BS b]
                pf = pfp.tile([128, BS], F32, tag="pf")
                for k in range(HC):
                    nc.tensor.matmul(pf, lhsT=wfc_chunk(k), rhs=h_out[:, k, :],
                                     start=(k == 0), stop=(k == HC - 1))
                nc.scalar.activation(in_out, pf, AF.Tanh, bias=fb_h, scale=0.5)

                # output fc in [b, i] layout for a 256B-row DMA (the [i, b]
                # layout halves descriptor size and measures ~8ms slower
                # end-to-end); bias via K=1 matmul
                po = pop.tile([BS, 128], F32, tag="po")
                for k in range(HC):
                    nc.tensor.matmul(po, lhsT=h_out[:, k, :], rhs=wfc_chunk(k),
                                     start=(k == 0), stop=False)
                nc.tensor.matmul(po, lhsT=ones[:, 0:BS], rhs=fb_r,
                                 start=False, stop=True)
                ob = outp.tile([BS, 128], F16, tag="ob")
                nc.scalar.activation(ob, po, AF.Tanh, scale=0.5)
                # repeat>1 (timing mode): extra steps clamp to row 0 (junk)
                row = (nsteps - 1 - t) * BS
                if repeat > 1:
                    row = smax(0, row)
                nc.sync.dma_start(out=out[ds(row, BS), :], in_=ob)

            unroll = 8 if (nsteps * repeat) % 8 == 0 else 2
            with tc.For_i(0, nsteps * repeat, unroll,
                          staggered_reset=True) as t:
                if unroll == 2:
                    step(t, hA, hB, ins[3], ins[0])
                    step(t + 1, hB, hA, ins[0], ins[3])
                else:
                    for u in range(0, unroll, 2):
                        step(t + u, hA, hB, ins[(u + 3) % 4], ins[u % 4])
                        step(t + u + 1, hB, hA, ins[u % 4],
                             ins[(u + 1) % 4])

    nc.finalize()
    return nc


_cache = {}


def _get_nc(nsteps, out_steps=None, repeat=1):
    key = (nsteps, out_steps, repeat)
    if key not in _cache:
        _cache[key] = build(nsteps, out_steps, repeat)
    return _cache[key]


def _prep_inputs(x, h0, c0, W_ih, W_hh, b_ih, b_hh, fc_W, fc_b, nsteps):
    """-> (shared_map {name: array}, percore_map {name: (8*128, cols) array})."""
    f32 = np.float32
    bf16 = ml_dtypes.bfloat16
    x = np.asarray(x, f32)
    h0 = np.asarray(h0, f32)
    c0 = np.asarray(c0, f32)
    # state is H=2h, C=2c with W_hh/fc_W halved to compensate; g-gate rows
    # doubled so all gates share tanh(0.5*(psum)) with psum pre-biased
    W_cat = np.concatenate(
        [np.asarray(W_ih, f32), 0.5 * np.asarray(W_hh, f32)], axis=1)
    W_cat[1024:1536, :] *= 2.0
    wg_np = W_cat.reshape(NM, 128, NK, 128).transpose(3, 0, 2, 1).reshape(
        128, NM * NK * 128)
    fc_W = np.asarray(fc_W, f32)
    wfc_np = (0.5 * fc_W).reshape(I, HC, 128).transpose(2, 1, 0).reshape(
        128, HC * 128)
    b = np.asarray(b_ih, f32) + np.asarray(b_hh, f32)
    badj = b.copy()
    badj[1024:1536] *= 2.0
    fc_b = np.asarray(fc_b, f32)

    wts = np.zeros((128, WT_COLS), f32)
    wts[:, OFF_WG:OFF_WG + NM * NK * 128] = wg_np
    wts[:, OFF_WFC:OFF_WFC + HC * 128] = wfc_np
    wts[0, OFF_FCBR:OFF_FCBR + 128] = fc_b
    # gate-bias lhsT [part=chunk c, col=j*128+p] = badj[j, c, p]
    wts[0:4, OFF_BMM:OFF_BMM + 4 * 128] = badj.reshape(
        4, HC, 128).transpose(1, 0, 2).reshape(HC, 4 * 128)
    wts[0:4, OFF_IND:OFF_IND + HC * BS] = np.repeat(np.eye(HC, dtype=f32),
                                                    BS, axis=1)

    bb = np.zeros((128, BB_COLS), f32)
    bb[:, OFF_FCBH] = 0.5 * fc_b

    # per-core bundles, stacked core-major for PartitionSpec("core")
    st = np.zeros((NCORES, 128, ST_COLS), f32)
    cc = np.zeros((NCORES, 128, C0_COLS), f32)
    for core in range(NCORES):
        sl = slice(core * BS, (core + 1) * BS)
        st[core, :, OFF_XT:OFF_XT + BS] = x[nsteps - 1, sl, :].T
        st[core, :, OFF_H0:OFF_H0 + HC * BS] = 2.0 * h0[0, sl, :].reshape(
            BS, HC, 128).transpose(2, 1, 0).reshape(128, -1)
        cc[core] = 2.0 * c0[0, sl, :].reshape(
            BS, HC, 128).transpose(2, 1, 0).reshape(128, -1)
    shared = {
        "wts16": np.ascontiguousarray(wts).astype(np.float16),
        "bb32": np.ascontiguousarray(bb),
    }
    percore = {
        "st16": np.ascontiguousarray(st.astype(np.float16)).reshape(
            NCORES * 128, ST_COLS),
        "c032": np.ascontiguousarray(cc).reshape(NCORES * 128, C0_COLS),
    }
    return shared, percore


class _Runner:
    """Cached jitted 8-core executor for one build() configuration.

    Bypasses run_bass_kernel_spmd so repeated calls reuse the jitted
    callable (no re-trace / NEFF reload) and so the donated output
    buffers are created ON DEVICE (jnp.zeros jit) instead of being
    shipped over the axon tunnel (134 MB of zeros at fp32).
    """

    def __init__(self, nsteps, out_steps=None, repeat=1):
        import jax
        import jax.numpy as jnp
        from jax.sharding import Mesh, PartitionSpec, NamedSharding
        from concourse.bass2jax import (
            _bass_exec_p, install_neuronx_cc_hook, partition_id_tensor,
            shard_map)

        self.jax = jax
        self.nsteps = nsteps
        self.out_steps = out_steps or nsteps
        nc = _get_nc(nsteps, out_steps, repeat)
        self.nc = nc
        install_neuronx_cc_hook()

        part_name = (nc.partition_id_tensor.name
                     if nc.partition_id_tensor else None)
        in_names, out_names, out_avals = [], [], []
        for alloc in nc.m.functions[0].allocations:
            if not isinstance(alloc, mybir.MemoryLocationSet):
                continue
            name = alloc.memorylocations[0].name
            if alloc.kind == "ExternalInput":
                if name != part_name:
                    in_names.append(name)
            elif alloc.kind == "ExternalOutput":
                out_names.append(name)
                out_avals.append(jax.core.ShapedArray(
                    tuple(alloc.tensor_shape), mybir.dt.np(alloc.dtype)))
        self.in_names = in_names
        self.out_names = out_names
        self.out_avals = out_avals
        self.shared_names = ("wts16", "bb32")
        n_params = len(in_names)
        n_outs = len(out_avals)
        all_names = tuple(in_names) + tuple(out_names)
        if part_name is not None:
            all_names = all_names + (part_name,)

        def _body(*args):
            operands = list(args)
            if part_name is not None:
                operands.append(partition_id_tensor())
            outs = _bass_exec_p.bind(
                *operands, out_avals=tuple(out_avals),
                in_names=all_names, out_names=tuple(out_names),
                lowering_input_output_aliases=(),
                sim_require_finite=True, sim_require_nnan=True, nc=nc)
            return tuple(outs)

        devices = jax.devices()[:NCORES]
        self.dev0 = devices[0]
        mesh = Mesh(np.asarray(devices), ("core",))
        self.sharding = NamedSharding(mesh, PartitionSpec("core"))
        self.rep_sharding = NamedSharding(mesh, PartitionSpec())
        in_specs = tuple(
            PartitionSpec() if n in self.shared_names else
            PartitionSpec("core") for n in in_names
        ) + (PartitionSpec("core"),) * n_outs
        out_specs = (PartitionSpec("core"),) * n_outs
        donate = tuple(range(n_params, n_params + n_outs))
        self.sharded = jax.jit(
            shard_map(_body, mesh=mesh, in_specs=in_specs,
                      out_specs=out_specs, check_rep=False),
            donate_argnums=donate, keep_unused=True)
        zero_shapes = [(NCORES * a.shape[0], *a.shape[1:]) for a in out_avals]
        zero_dtypes = [a.dtype for a in out_avals]
        self.mkzeros = jax.jit(
            lambda: tuple(jnp.zeros(s, d)
                          for s, d in zip(zero_shapes, zero_dtypes)),
            out_shardings=tuple(self.sharding for _ in out_avals))

    def device_put_inputs(self, shared, percore):
        """Upload: shared tensors once + device-side broadcast; per-core
        tensors sharded. Returns the operand list in in_names order."""
        jax = self.jax
        arrs = []
        for name in self.in_names:
            if name in self.shared_names:
                d0 = jax.device_put(shared[name],
                                    jax.sharding.SingleDeviceSharding(
                                        self.dev0))
                arrs.append(jax.device_put(d0, self.rep_sharding))
            else:
                arrs.append(jax.device_put(percore[name], self.sharding))
        jax.block_until_ready(arrs)
        return arrs

    def execute(self, operands):
        outs = self.sharded(*operands, *self.mkzeros())
        self.jax.block_until_ready(outs)
        return outs

    def fetch(self, outs):
        return {name: np.asarray(o) for name, o in zip(self.out_names, outs)}


def jax_block(x):
    import jax
    jax.block_until_ready(x)


_runners = {}


def _get_runner(nsteps, out_steps=None, repeat=1) -> _Runner:
    key = (nsteps, out_steps, repeat)
    if key not in _runners:
        _runners[key] = _Runner(nsteps, out_steps, repeat)
    return _runners[key]


def _assemble(out_global: np.ndarray, steps: int) -> np.ndarray:
    """(8*steps*BS, I) core-major device output -> (steps, B, I) fp32."""
    per_core = out_global.reshape(NCORES, steps, BS, I)
    return per_core.transpose(1, 0, 2, 3).reshape(
        steps, B, I).astype(np.float32)


def run(x, h0, c0, W_ih, W_hh, b_ih, b_hh, fc_W, fc_b, nsteps=T,
        out_steps=None, repeat=1):
    r = _get_runner(nsteps, out_steps, repeat)
    shared, percore = _prep_inputs(x, h0, c0, W_ih, W_hh, b_ih, b_hh,
                                   fc_W, fc_b, nsteps)
    outs = r.execute(r.device_put_inputs(shared, percore))
    host = r.fetch(outs)
    return _assemble(host["out"], out_steps or nsteps), None


def kernel(x, enc_hiddens, h0, c0, W_ih, W_hh, b_ih, b_hh, fc_W, fc_b):
    outs, _ = run(x, h0, c0, W_ih, W_hh, b_ih, b_hh, fc_W, fc_b, nsteps=T)
    return outs



# revision 61
# speedup vs baseline: 1.1376x; 1.0019x over previous
"""Recursive LSTM decoder (T=512, B=512, I=128, H=512) on 8 trn2 NeuronCores.

Strategy: data-parallel over batch (64 rows/core, weights replicated, no
collectives). All on-chip state is kept in transposed layout
[feature-on-partition, batch-on-free] so the serial recurrence needs no
transposes. Matmul inputs are fp16 (1 cycle/row on PE, 10 mantissa bits);
accumulation and elementwise math are fp32; the cell state c stays fp32.

Per step (per core), gate order f, i, g, o:
  psg[j] = bias (K=4 matmul vs a chunk indicator, start=True)
           + sum_k Wcat.T-chunk(k,m) @ catT-chunk(k)  (16 m x 5 k, N=64)
  tanh-only gates: sg(z)=(tanh(z/2)+1)/2 with state C=2c, H=2h (W_hh/fc_W
  host-halved, g-gate rows host-doubled), so every gate is tanh(0.5 psum)
  and the c/h chain is 3 DVE stt ops + 1 ACT tanh, half-split so the
  ACT->DVE chain pipelines and mostly hides under the o-gate matmuls.
  feedback: in' = tanh(0.5*(fcW.T @ hT) + fc_b/2)   [= 2*sigmoid(z)-1]
  output:   out[64,128] = tanh(0.5*(hT stationary @ fcW-moving + fc_b)),
  stored fp16 at DRAM row (T-1-t)*BS (reference stores outputs reversed).

The loop is unrolled 8 steps per For_i iteration to amortize the
staggered_reset semaphore barrier; 4 rotating feedback buffers give each
output-store DMA 4 steps of slack. Host<->device traffic is minimized:
weights/biases upload once and broadcast device-side (replicated
PartitionSpec), per-core state is sharded, donated output buffers are
created on-device, and the output downloads as fp16.
"""

import numpy as np

import concourse.mybir as mybir
import concourse.tile as tile
from concourse import bacc
from concourse.bass import ds
from concourse.expressions import smax

T, B, I, H = 512, 512, 128, 512
NCORES = 8
BS = B // NCORES          # 64 batch rows per core
HC = H // 128             # 4 h chunks
NM = (4 * H) // 128       # 16 gate m-chunks
NK = (I + H) // 128       # 5 cat k-chunks (1 input + 4 hidden)

# fp16 WEIGHT bundle (identical on every core -> replicated upload)
OFF_WG = 0                       # [128, NM*NK*128] gate weight chunks
OFF_WFC = OFF_WG + NM * NK * 128  # [128, HC*128] fc weight chunks
OFF_FCBR = OFF_WFC + HC * 128    # [1, 128] fc bias row (row 0 only)
OFF_BMM = OFF_FCBR + 128         # [4, 4*128] gate bias lhsT (part=chunk c)
OFF_IND = OFF_BMM + 4 * 128      # [4, HC*BS] chunk indicator rhs
WT_COLS = OFF_IND + HC * BS
# fp16 per-core STATE bundle
OFF_XT = 0                       # [128, BS] x[T-1] transposed
OFF_H0 = OFF_XT + BS             # [128, HC*BS] h0 transposed
ST_COLS = OFF_H0 + HC * BS
# f32 BIAS bundle (identical on every core -> replicated upload)
OFF_FCBH = 0                     # [128, 1] fc_b / 2
BB_COLS = OFF_FCBH + 1
# f32 per-core c0 bundle: [128, HC*BS]
C0_COLS = HC * BS

BF16 = mybir.dt.bfloat16
F16 = mybir.dt.float16
F32 = mybir.dt.float32
AF = mybir.ActivationFunctionType


def build(nsteps: int, out_steps: int | None = None, repeat: int = 1):
    """repeat>1 is a timing mode: the loop runs nsteps*repeat steps; stores
    for t >= nsteps fall out of range and are skipped via bounds_check."""
    out_steps = out_steps or nsteps
    nc = bacc.Bacc()
    wts16 = nc.dram_tensor("wts16", [128, WT_COLS], F16, kind="ExternalInput")
    st16 = nc.dram_tensor("st16", [128, ST_COLS], F16, kind="ExternalInput")
    bb32 = nc.dram_tensor("bb32", [128, BB_COLS], F32, kind="ExternalInput")
    c032 = nc.dram_tensor("c032", [128, C0_COLS], F32, kind="ExternalInput")
    out = nc.dram_tensor("out", [out_steps * BS, I], F16,
                         kind="ExternalOutput")

    with tile.TileContext(nc) as tc:
        with (
            tc.tile_pool(name="consts", bufs=1) as consts,
            tc.tile_pool(name="state", bufs=1) as state,
            tc.tile_pool(name="gact", bufs=3) as gact,
            tc.tile_pool(name="outp", bufs=3) as outp,
            tc.tile_pool(name="psst", bufs=1, space="PSUM") as psst,
            tc.tile_pool(name="pf", bufs=2, space="PSUM") as pfp,
            tc.tile_pool(name="po", bufs=2, space="PSUM") as pop,
        ):
            CW = consts.tile([128, WT_COLS], F16)
            nc.sync.dma_start(out=CW, in_=wts16[:])
            CS = consts.tile([128, ST_COLS], F16)
            nc.sync.dma_start(out=CS, in_=st16[:])
            CB = consts.tile([128, BB_COLS], F32)
            nc.sync.dma_start(out=CB, in_=bb32[:])
            CC = consts.tile([128, C0_COLS], F32)
            nc.sync.dma_start(out=CC, in_=c032[:])
            ones = consts.tile([1, HC * BS], F16)
            nc.vector.memset(ones, 1.0)

            def wg_chunk(m, k):
                o = OFF_WG + (m * NK + k) * 128
                return CW[:, o:o + 128]

            def wfc_chunk(k):
                o = OFF_WFC + k * 128
                return CW[:, o:o + 128]

            fb_r = CW[0:1, OFF_FCBR:OFF_FCBR + 128]
            bias_mm = CW[0:4, OFF_BMM:OFF_BMM + 4 * 128]
            ind_mm = CW[0:4, OFF_IND:OFF_IND + HC * BS]
            fb_h = CB[:, OFF_FCBH:OFF_FCBH + 1]

            hA = state.tile([128, HC, BS], F16)
            nc.vector.tensor_copy(
                hA, CS[:, OFF_H0:OFF_H0 + HC * BS].rearrange(
                    "p (c b) -> p c b", c=HC))
            hB = state.tile([128, HC, BS], F16)
            cT = state.tile([128, HC, BS], F32)
            nc.vector.tensor_copy(
                cT, CC[:, 0:HC * BS].rearrange("p (c b) -> p c b", c=HC))
            # 4 rotating feedback/output buffers: step u reads ins[(u+3)%4],
            # writes ins[u%4]; the store DMA of a buffer gets 4 steps of
            # slack before the buffer is rewritten. x[T-1] seeds ins[3].
            ins = [state.tile([128, BS], F16, name=f"in{q}")
                   for q in range(4)]
            nc.vector.tensor_copy(ins[3], CS[:, OFF_XT:OFF_XT + BS])
            # prologue tanh so the ACT table set is loaded on every path into
            # the loop -- otherwise the table-load lands INSIDE the body
            warm = state.tile([128, 1], F32)
            nc.scalar.activation(warm, CB[:, OFF_FCBH:OFF_FCBH + 1], AF.Tanh)

            # persistent per-gate PSUM accumulators [p, h-chunk, b]; each
            # step's first write is a start=True K=4 bias matmul (chunk
            # indicator x per-chunk bias row), so no DVE seeding needed
            psg = [psst.tile([128, HC, BS], F32, name=f"psg{j}")
                   for j in range(4)]

            cTf = cT.rearrange("p c b -> p (c b)")
            psgf = [p.rearrange("p c b -> p (c b)") for p in psg]

            def step(t, h_in, h_out, in_in, in_out):
                # Per-gate PSUM: psg[j] holds gate j for all 4 H-chunks.
                # A K=4 bias matmul (start=True) seeds the bank; the gate
                # matmuls accumulate on top (start=False).
                # Gate order f, i, g, o: u_s=(th_f+1)*C can run right after
                # the first ACT, and the v_s/cTf/tanh_c chain hides under
                # the o-gate matmuls; only tanh_o -> h remains in the tail.
                # sigmoid-free: sg(z)=(tanh(z/2)+1)/2, state C=2c, H=2h
                # (W_hh, fc_W host-halved; g-gate weights/bias host-doubled
                # so every gate uses tanh(0.5*psum)).
                th = {}
                v_s = gact.tile([128, HC * BS], F32, tag="v_s")
                u_s = gact.tile([128, HC * BS], F32, tag="u_s")
                tc_s = gact.tile([128, HC * BS], F32, tag="tc_s")
                HB2 = HC * BS // 2  # half-split of the c/h chain ops

                def half(x, s):
                    return x[:, s * HB2:(s + 1) * HB2]

                for j in (1, 0, 2, 3):
                    nc.tensor.matmul(
                        psgf[j], lhsT=bias_mm[:, j * 128:(j + 1) * 128],
                        rhs=ind_mm, start=True, stop=False,
                        skip_group_check=True)
                    for c in range(HC):
                        m = j * 4 + c
                        for k in (1, 2, 3, 4, 0):
                            mv = in_in if k == 0 else h_in[:, k - 1, :]
                            nc.tensor.matmul(
                                psg[j][:, c, :], lhsT=wg_chunk(m, k), rhs=mv,
                                start=False, stop=(k == 0),
                                skip_group_check=True)
                    th_j = gact.tile([128, HC * BS], F32, tag=f"th{j}")
                    th[j] = th_j
                    if j in (2, 3):
                        # on the critical chain: half-split ACT so the DVE
                        # consumers pipeline behind the first half
                        for s in (0, 1):
                            nc.scalar.activation(half(th_j, s),
                                                 half(psgf[j], s),
                                                 AF.Tanh, scale=0.5)
                    else:
                        nc.scalar.activation(th_j, psgf[j], AF.Tanh,
                                             scale=0.5)
                    if j == 1:
                        # A=(th_f+1)*C=4fc
                        nc.vector.scalar_tensor_tensor(
                            u_s, th[1], 1.0, cTf,
                            op0=mybir.AluOpType.add, op1=mybir.AluOpType.mult)
                    elif j == 2:
                        # B=(th_i+1)*g=2ig, C_new=A/2+B=2c_new
                        for s in (0, 1):
                            nc.vector.scalar_tensor_tensor(
                                half(v_s, s), half(th[0], s), 1.0,
                                half(th[2], s),
                                op0=mybir.AluOpType.add,
                                op1=mybir.AluOpType.mult)
                            nc.vector.scalar_tensor_tensor(
                                half(cTf, s), half(u_s, s), 0.5,
                                half(v_s, s),
                                op0=mybir.AluOpType.mult,
                                op1=mybir.AluOpType.add)
                            nc.scalar.activation(half(tc_s, s),
                                                 half(cTf, s),
                                                 AF.Tanh, scale=0.5)
                # H = (th_o+1)*tanh(c) = 2h
                hof = h_out.rearrange("p c b -> p (c b)")
                for s in (0, 1):
                    nc.vector.scalar_tensor_tensor(
                        half(hof, s), half(th[3], s), 1.0, half(tc_s, s),
                        op0=mybir.AluOpType.add, op1=mybir.AluOpType.mult)

                # feedback fc: in_out = tanh(0.5*fc(h) + fc_b/2) [128 i, BS b]
                pf = pfp.tile([128, BS], F32, tag="pf")
                for k in range(HC):
                    nc.tensor.matmul(pf, lhsT=wfc_chunk(k), rhs=h_out[:, k, :],
                                     start=(k == 0), stop=(k == HC - 1))
                nc.scalar.activation(in_out, pf, AF.Tanh, bias=fb_h, scale=0.5)

                # output fc in [b, i] layout for a 256B-row DMA (the [i, b]
                # layout halves descriptor size and measures ~8ms slower
                # end-to-end); bias via K=1 matmul
                po = pop.tile([BS, 128], F32, tag="po")
                for k in range(HC):
                    nc.tensor.matmul(po, lhsT=h_out[:, k, :], rhs=wfc_chunk(k),
                                     start=(k == 0), stop=False)
                nc.tensor.matmul(po, lhsT=ones[:, 0:BS], rhs=fb_r,
                                 start=False, stop=True)
                ob = outp.tile([BS, 128], F16, tag="ob")
                nc.scalar.activation(ob, po, AF.Tanh, scale=0.5)
                # repeat>1 (timing mode): extra steps clamp to row 0 (junk)
                row = (nsteps - 1 - t) * BS
                if repeat > 1:
                    row = smax(0, row)
                nc.sync.dma_start(out=out[ds(row, BS), :], in_=ob)

            unroll = 8 if (nsteps * repeat) % 8 == 0 else 2
            with tc.For_i(0, nsteps * repeat, unroll,
                          staggered_reset=True) as t:
                if unroll == 2:
                    step(t, hA, hB, ins[3], ins[0])
                    step(t + 1, hB, hA, ins[0], ins[3])
                else:
                    for u in range(0, unroll, 2):
                        step(t + u, hA, hB, ins[(u + 3) % 4], ins[u % 4])
                        step(t + u + 1, hB, hA, ins[u % 4],
                             ins[(u + 1) % 4])

    nc.finalize()
    return nc


_cache = {}


def _get_nc(nsteps, out_steps=None, repeat=1):
    key = (nsteps, out_steps, repeat)
    if key not in _cache:
        _cache[key] = build(nsteps, out_steps, repeat)
    return _cache[key]


def _prep_inputs(x, h0, c0, W_ih, W_hh, b_ih, b_hh, fc_W, fc_b, nsteps):
    """-> (shared_map {name: array}, percore_map {name: (8*128, cols) array})."""
    f32 = np.float32
    x = np.asarray(x, f32)
    h0 = np.asarray(h0, f32)
    c0 = np.asarray(c0, f32)
    # state is H=2h, C=2c with W_hh/fc_W halved to compensate; g-gate rows
    # doubled so all gates share tanh(0.5*(psum)) with psum pre-biased
    W_cat = np.concatenate(
        [np.asarray(W_ih, f32), 0.5 * np.asarray(W_hh, f32)], axis=1)
    W_cat[1024:1536, :] *= 2.0
    wg_np = W_cat.reshape(NM, 128, NK, 128).transpose(3, 0, 2, 1).reshape(
        128, NM * NK * 128)
    fc_W = np.asarray(fc_W, f32)
    wfc_np = (0.5 * fc_W).reshape(I, HC, 128).transpose(2, 1, 0).reshape(
        128, HC * 128)
    b = np.asarray(b_ih, f32) + np.asarray(b_hh, f32)
    badj = b.copy()
    badj[1024:1536] *= 2.0
    fc_b = np.asarray(fc_b, f32)

    wts = np.zeros((128, WT_COLS), f32)
    wts[:, OFF_WG:OFF_WG + NM * NK * 128] = wg_np
    wts[:, OFF_WFC:OFF_WFC + HC * 128] = wfc_np
    wts[0, OFF_FCBR:OFF_FCBR + 128] = fc_b
    # gate-bias lhsT [part=chunk c, col=j*128+p] = badj[j, c, p]
    wts[0:4, OFF_BMM:OFF_BMM + 4 * 128] = badj.reshape(
        4, HC, 128).transpose(1, 0, 2).reshape(HC, 4 * 128)
    wts[0:4, OFF_IND:OFF_IND + HC * BS] = np.repeat(np.eye(HC, dtype=f32),
                                                    BS, axis=1)

    bb = np.zeros((128, BB_COLS), f32)
    bb[:, OFF_FCBH] = 0.5 * fc_b

    # per-core bundles, stacked core-major for PartitionSpec("core")
    st = np.zeros((NCORES, 128, ST_COLS), f32)
    cc = np.zeros((NCORES, 128, C0_COLS), f32)
    for core in range(NCORES):
        sl = slice(core * BS, (core + 1) * BS)
        st[core, :, OFF_XT:OFF_XT + BS] = x[nsteps - 1, sl, :].T
        st[core, :, OFF_H0:OFF_H0 + HC * BS] = 2.0 * h0[0, sl, :].reshape(
            BS, HC, 128).transpose(2, 1, 0).reshape(128, -1)
        cc[core] = 2.0 * c0[0, sl, :].reshape(
            BS, HC, 128).transpose(2, 1, 0).reshape(128, -1)
    shared = {
        "wts16": np.ascontiguousarray(wts).astype(np.float16),
        "bb32": np.ascontiguousarray(bb),
    }
    percore = {
        "st16": np.ascontiguousarray(st.astype(np.float16)).reshape(
            NCORES * 128, ST_COLS),
        "c032": np.ascontiguousarray(cc).reshape(NCORES * 128, C0_COLS),
    }
    return shared, percore


class _Runner:
    """Cached jitted 8-core executor for one build() configuration.

    Bypasses run_bass_kernel_spmd so repeated calls reuse the jitted
    callable (no re-trace / NEFF reload) and so the donated output
    buffers are created ON DEVICE (jnp.zeros jit) instead of being
    shipped over the axon tunnel (134 MB of zeros at fp32).
    """

    def __init__(self, nsteps, out_steps=None, repeat=1):
        import jax
        import jax.numpy as jnp
        from jax.sharding import Mesh, PartitionSpec, NamedSharding
        from concourse.bass2jax import (
            _bass_exec_p, install_neuronx_cc_hook, partition_id_tensor,
            shard_map)

        self.jax = jax
        self.nsteps = nsteps
        self.out_steps = out_steps or nsteps
        nc = _get_nc(nsteps, out_steps, repeat)
        self.nc = nc
        install_neuronx_cc_hook()

        part_name = (nc.partition_id_tensor.name
                     if nc.partition_id_tensor else None)
        in_names, out_names, out_avals = [], [], []
        for alloc in nc.m.functions[0].allocations:
            if not isinstance(alloc, mybir.MemoryLocationSet):
                continue
            name = alloc.memorylocations[0].name
            if alloc.kind == "ExternalInput":
                if name != part_name:
                    in_names.append(name)
            elif alloc.kind == "ExternalOutput":
                out_names.append(name)
                out_avals.append(jax.core.ShapedArray(
                    tuple(alloc.tensor_shape), mybir.dt.np(alloc.dtype)))
        self.in_names = in_names
        self.out_names = out_names
        self.out_avals = out_avals
        self.shared_names = ("wts16", "bb32")
        n_params = len(in_names)
        n_outs = len(out_avals)
        all_names = tuple(in_names) + tuple(out_names)
        if part_name is not None:
            all_names = all_names + (part_name,)

        def _body(*args):
            operands = list(args)
            if part_name is not None:
                operands.append(partition_id_tensor())
            outs = _bass_exec_p.bind(
                *operands, out_avals=tuple(out_avals),
                in_names=all_names, out_names=tuple(out_names),
                lowering_input_output_aliases=(),
                sim_require_finite=True, sim_require_nnan=True, nc=nc)
            return tuple(outs)

        devices = jax.devices()[:NCORES]
        self.dev0 = devices[0]
        mesh = Mesh(np.asarray(devices), ("core",))
        self.sharding = NamedSharding(mesh, PartitionSpec("core"))
        self.rep_sharding = NamedSharding(mesh, PartitionSpec())
        in_specs = tuple(
            PartitionSpec() if n in self.shared_names else
            PartitionSpec("core") for n in in_names
        ) + (PartitionSpec("core"),) * n_outs
        out_specs = (PartitionSpec("core"),) * n_outs
        donate = tuple(range(n_params, n_params + n_outs))
        self.sharded = jax.jit(
            shard_map(_body, mesh=mesh, in_specs=in_specs,
                      out_specs=out_specs, check_rep=False),
            donate_argnums=donate, keep_unused=True)
        zero_shapes = [(NCORES * a.shape[0], *a.shape[1:]) for a in out_avals]
        zero_dtypes = [a.dtype for a in out_avals]
        self.mkzeros = jax.jit(
            lambda: tuple(jnp.zeros(s, d)
                          for s, d in zip(zero_shapes, zero_dtypes)),
            out_shardings=tuple(self.sharding for _ in out_avals))

    def device_put_inputs(self, shared, percore):
        """Upload: shared tensors once + device-side broadcast; per-core
        tensors sharded. Returns the operand list in in_names order."""
        jax = self.jax
        arrs = []
        for name in self.in_names:
            if name in self.shared_names:
                d0 = jax.device_put(shared[name],
                                    jax.sharding.SingleDeviceSharding(
                                        self.dev0))
                arrs.append(jax.device_put(d0, self.rep_sharding))
            else:
                arrs.append(jax.device_put(percore[name], self.sharding))
        jax.block_until_ready(arrs)
        return arrs

    def execute(self, operands):
        outs = self.sharded(*operands, *self.mkzeros())
        self.jax.block_until_ready(outs)
        return outs

    def fetch(self, outs):
        return {name: np.asarray(o) for name, o in zip(self.out_names, outs)}


def jax_block(x):
    import jax
    jax.block_until_ready(x)


_runners = {}


def _get_runner(nsteps, out_steps=None, repeat=1) -> _Runner:
    key = (nsteps, out_steps, repeat)
    if key not in _runners:
        _runners[key] = _Runner(nsteps, out_steps, repeat)
    return _runners[key]


def _assemble(out_global: np.ndarray, steps: int) -> np.ndarray:
    """(8*steps*BS, I) core-major device output -> (steps, B, I) fp32."""
    per_core = out_global.reshape(NCORES, steps, BS, I)
    return per_core.transpose(1, 0, 2, 3).reshape(
        steps, B, I).astype(np.float32)


def run(x, h0, c0, W_ih, W_hh, b_ih, b_hh, fc_W, fc_b, nsteps=T,
        out_steps=None, repeat=1):
    r = _get_runner(nsteps, out_steps, repeat)
    shared, percore = _prep_inputs(x, h0, c0, W_ih, W_hh, b_ih, b_hh,
                                   fc_W, fc_b, nsteps)
    outs = r.execute(r.device_put_inputs(shared, percore))
    host = r.fetch(outs)
    return _assemble(host["out"], out_steps or nsteps), None


def kernel(x, enc_hiddens, h0, c0, W_ih, W_hh, b_ih, b_hh, fc_W, fc_b):
    outs, _ = run(x, h0, c0, W_ih, W_hh, b_ih, b_hh, fc_W, fc_b, nsteps=T)
    return outs

